# revision 7
# baseline (speedup 1.0000x reference)
"""Invariant particle attention (Lorentz-frame SDPA) on 8 trn2 cores.

Full-input contract: kernel(**inputs) takes the unsharded tensors
  q_local/k_local/v_local [B=8, H=8, N=1024, C=128] f32,
  lframes_matrices        [B=8, N=1024, 4, 4]      f32
and returns out [B, H, N, C] f32.

Sharding: data-parallel over batch B — core b processes batch b
(attention mixes only within a batch element; no collectives).

Per-core pipeline (one batch, all 8 heads):
  phase 1: load q/k/v n-tiles (particles on partitions, all heads wide),
           apply per-particle 4x4 Lorentz mixes on DVE
           (16 fused scalar*tensor+tensor ops per tensor per tile),
           PE-transpose q_g/k_g to channel-major [C, N]; v_g stays
           natural [N, C] in bf16 with a ones column appended.
  phase 2: per head: S^T[j,i] = k_gT(j-tile)^T . q_gT  (fp32r matmuls,
           full rate at N=512), exp((1/sqrt(C)) S) on ACT -> P bf16;
           U[i, 0:129] = sum_j P^T-slice^T . [v_g | 1]  (bf16 matmuls,
           PSUM accumulation) -> col 128 is the softmax denominator.
  phase 3: reciprocal of denominators, apply Lambda mix to U across all
           heads at once, scale by reciprocal, store.

build_nc(repeat=K) wraps the body in a hardware For-loop executing the
same computation K times — used only for timing (per-exec time =
(T(K) - T(1)) / (K - 1), cancelling dispatch/tunnel overhead).
"""

import contextlib
import sys

if "/opt/trn_rl_repo" not in sys.path:
    sys.path.insert(0, "/opt/trn_rl_repo")

import numpy as np

import concourse.bass as bass
import concourse.mybir as mybir
import concourse.tile as tile
from concourse import bacc
from concourse.bass import ts
from concourse.bass_utils import run_bass_kernel_spmd
from concourse.masks import make_identity

F32 = mybir.dt.float32
F32R = mybir.dt.float32r
BF16 = mybir.dt.bfloat16
MULT = mybir.AluOpType.mult
ADD = mybir.AluOpType.add

NS = 8  # scalar channels per head
NV = 30  # four-vector channels per head
P = 128  # partition tile (particles)


def build_nc(H=8, N=1024, C=128, repeat=1):
    NT = N // P
    scale = 1.0 / float(np.sqrt(C))

    nc = bacc.Bacc("TRN2", target_bir_lowering=False, debug=False)
    q_d = nc.dram_tensor("q", [H, N, C], F32, kind="ExternalInput")
    k_d = nc.dram_tensor("k", [H, N, C], F32, kind="ExternalInput")
    v_d = nc.dram_tensor("v", [H, N, C], F32, kind="ExternalInput")
    # per-particle 4x4 matrices, row-major 16 entries each
    inv_d = nc.dram_tensor("invE", [N, 16], F32, kind="ExternalInput")
    linv_d = nc.dram_tensor("linvE", [N, 16], F32, kind="ExternalInput")
    lam_d = nc.dram_tensor("lamE", [N, 16], F32, kind="ExternalInput")
    out_d = nc.dram_tensor("out", [H, N, C], F32, kind="ExternalOutput")

    q_r = q_d.ap().rearrange("h (t p) c -> t p h c", p=P)
    k_r = k_d.ap().rearrange("h (t p) c -> t p h c", p=P)
    v_r = v_d.ap().rearrange("h (t p) c -> t p h c", p=P)
    out_r = out_d.ap().rearrange("h (t p) c -> t p h c", p=P)
    inv_r = inv_d.ap().rearrange("(t p) e -> t p e", p=P)
    linv_r = linv_d.ap().rearrange("(t p) e -> t p e", p=P)
    lam_r = lam_d.ap().rearrange("(t p) e -> t p e", p=P)

    # vec-channel view helper: [..., NS:C] -> [..., 4, NV]
    def vecview(ap):  # ap [..., C]
        return ap[..., NS:C].rearrange("... (v f) -> ... f v", f=4)

    with tile.TileContext(nc) as tc:
        with (
            tc.tile_pool(name="singles", bufs=1) as singles,
            tc.tile_pool(name="persist", bufs=1) as persist,
            tc.tile_pool(name="stage", bufs=2) as stage,
            tc.tile_pool(name="gbuf", bufs=1) as gbuf,
            tc.tile_pool(name="ent", bufs=2) as ent,
            tc.tile_pool(name="pbuf", bufs=2) as pbuf,
            tc.tile_pool(name="obuf", bufs=1) as obuf,
            tc.tile_pool(name="tps", bufs=2, space="PSUM") as tps_pool,
            tc.tile_pool(name="sps", bufs=2, space="PSUM") as sps_pool,
            tc.tile_pool(name="ups", bufs=2, space="PSUM") as ups_pool,
        ):
            idt = singles.tile([P, P], F32)
            make_identity(nc, idt)

            loop_ctx = (
                tc.For_i(
                    0,
                    repeat,
                    1,
                    hint_engines=(
                        mybir.EngineType.PE,
                        mybir.EngineType.DVE,
                        mybir.EngineType.Activation,
                        mybir.EngineType.SP,
                        mybir.EngineType.Pool,
                    ),
                )
                if repeat > 1
                else contextlib.nullcontext()
            )
            with loop_ctx:
                # persistent buffers
                qgT = persist.tile([P, H, N], F32R, tag="qgT")  # [c, h, n]
                kgT = persist.tile([P, H, N], F32R, tag="kgT")
                vg = persist.tile([P, NT, H, 132], BF16, tag="vg")
                usb = persist.tile([P, NT, H, C], F32, tag="usb")  # [i, t, h, c]
                rsb = persist.tile([P, NT, H], F32, tag="rsb")  # denominators
                rcp = persist.tile([P, NT, H], F32, tag="rcp")

                # ---------- phase 1: transforms + transposes ----------
                for t in range(NT):
                    qs = stage.tile([P, 3, H, C], F32, tag="qs")
                    nc.sync.dma_start(out=qs[:, 0], in_=q_r[t])
                    nc.sync.dma_start(out=qs[:, 1], in_=k_r[t])
                    nc.sync.dma_start(out=qs[:, 2], in_=v_r[t])
                    invT = ent.tile([P, 16], F32, tag="invT")
                    linvT = ent.tile([P, 16], F32, tag="linvT")
                    nc.sync.dma_start(out=invT, in_=inv_r[t])
                    nc.sync.dma_start(out=linvT, in_=linv_r[t])

                    gq = gbuf.tile([P, H, C], F32, tag="gq")
                    gk = gbuf.tile([P, H, C], F32, tag="gk")

                    qs_vec = vecview(qs)  # [P, 3, H, 4, NV]
                    gq_vec = vecview(gq)  # [P, H, 4, NV]
                    gk_vec = vecview(gk)
                    vg_t = vg[:, t]  # [P, H, 132]
                    vg_vec = vecview(vg_t[:, :, 0:C])  # [P, H, 4, NV]

                    for mu in range(4):
                        for x, (g_out, m_t) in enumerate(
                            [(gq_vec, invT), (gk_vec, linvT), (vg_vec, invT)]
                        ):
                            o = g_out[:, :, mu, :]
                            for nu in range(4):
                                col = m_t[:, 4 * mu + nu : 4 * mu + nu + 1]
                                i0 = qs_vec[:, x, :, nu, :]
                                if nu == 0:
                                    nc.vector.tensor_scalar_mul(o, i0, col)
                                else:
                                    nc.vector.scalar_tensor_tensor(
                                        out=o, in0=i0, scalar=col, in1=o,
                                        op0=MULT, op1=ADD,
                                    )
                    # scalar channels pass through
                    nc.any.tensor_copy(gq[:, :, 0:NS], qs[:, 0, :, 0:NS])
                    nc.any.tensor_copy(gk[:, :, 0:NS], qs[:, 1, :, 0:NS])
                    nc.any.tensor_copy(vg_t[:, :, 0:NS], qs[:, 2, :, 0:NS])
                    nc.vector.memset(vg_t[:, :, C : C + 1], 1.0)

                    # PE transposes: [n-tile, c] -> [c, n-tile] per head
                    for h in range(H):
                        pq = tps_pool.tile([P, P], F32, tag="tp")
                        nc.tensor.transpose(pq, gq[:, h, :], idt)
                        nc.any.tensor_copy(qgT[:, h, ts(t, P)], pq)
                        pk = tps_pool.tile([P, P], F32, tag="tp")
                        nc.tensor.transpose(pk, gk[:, h, :], idt)
                        nc.any.tensor_copy(kgT[:, h, ts(t, P)], pk)

                # ---------- phase 2: attention per head ----------
                n_half = min(512, N)
                for h in range(H):
                    pexp = pbuf.tile([P, NT, N], BF16, tag="pexp")  # [j, tj, i]
                    for jt in range(NT):
                        sT = sps_pool.tile([P, N], F32, tag="sT")
                        lhs = kgT[:, h, ts(jt, P)]
                        for half in range(N // n_half):
                            nc.tensor.matmul(
                                sT[:, ts(half, n_half)],
                                lhs,
                                qgT[:, h, ts(half, n_half)],
                                start=True, stop=True,
                            )
                        nc.scalar.activation(
                            pexp[:, jt, :], sT,
                            mybir.ActivationFunctionType.Exp, scale=scale,
                        )
                    for it in range(NT):
                        ups = ups_pool.tile([P, C + 1], F32, tag="ups")
                        for jt in range(NT):
                            nc.tensor.matmul(
                                ups,
                                pexp[:, jt, ts(it, P)],
                                vg[:, jt, h, 0 : C + 1],
                                start=(jt == 0), stop=(jt == NT - 1),
                            )
                        nc.any.tensor_copy(usb[:, it, h, :], ups[:, 0:C])
                        nc.any.tensor_copy(rsb[:, it, h : h + 1], ups[:, C : C + 1])

                # ---------- phase 3: output transform ----------
                for it in range(NT):
                    nc.vector.reciprocal(rcp[:, it, :], rsb[:, it, :])
                    lamT = ent.tile([P, 16], F32, tag="lamT")
                    nc.sync.dma_start(out=lamT, in_=lam_r[it])
                    osb = obuf.tile([P, H, C], F32, tag="osb")
                    u_vec = vecview(usb[:, it])  # [P, H, 4, NV]
                    o_vec = vecview(osb)
                    for mu in range(4):
                        o = o_vec[:, :, mu, :]
                        for nu in range(4):
                            col = lamT[:, 4 * mu + nu : 4 * mu + nu + 1]
                            i0 = u_vec[:, :, nu, :]
                            if nu == 0:
                                nc.vector.tensor_scalar_mul(o, i0, col)
                            else:
                                nc.vector.scalar_tensor_tensor(
                                    out=o, in0=i0, scalar=col, in1=o,
                                    op0=MULT, op1=ADD,
                                )
                    nc.any.tensor_copy(osb[:, :, 0:NS], usb[:, it, :, 0:NS])
                    # normalize: multiply all C channels by 1/rowsum per head
                    for h in range(H):
                        nc.vector.tensor_scalar_mul(
                            osb[:, h, :], osb[:, h, :], rcp[:, it, h : h + 1]
                        )
                    nc.sync.dma_start(out=out_r[it], in_=osb)

    nc.compile()
    return nc


def lorentz_entries(L):
    """L [..., 4, 4] -> (invE, linvE, lamE) each [..., 16] f32.

    inv = eta L^T eta ; lower_inv = L^T eta ; lam = L.
    """
    s = np.array([1.0, -1.0, -1.0, -1.0], dtype=np.float32)
    LT = np.swapaxes(L, -1, -2)
    invE = (s[:, None] * s[None, :]) * LT
    linvE = LT * s[None, :]
    sh = L.shape[:-2] + (16,)
    return (
        np.ascontiguousarray(invE, dtype=np.float32).reshape(sh),
        np.ascontiguousarray(linvE, dtype=np.float32).reshape(sh),
        np.ascontiguousarray(L, dtype=np.float32).reshape(sh),
    )


_NC_CACHE = {}


def kernel(q_local, k_local, v_local, lframes_matrices, _results_hook=None):
    B, H, N, C = q_local.shape
    assert (B, H, N, C) == (8, 8, 1024, 128), (B, H, N, C)

    if "nc" not in _NC_CACHE:
        _NC_CACHE["nc"] = build_nc(H=H, N=N, C=C)
    nc = _NC_CACHE["nc"]

    invE, linvE, lamE = lorentz_entries(np.asarray(lframes_matrices))
    in_maps = []
    for b in range(B):
        in_maps.append(
            {
                "q": np.ascontiguousarray(q_local[b], dtype=np.float32),
                "k": np.ascontiguousarray(k_local[b], dtype=np.float32),
                "v": np.ascontiguousarray(v_local[b], dtype=np.float32),
                "invE": invE[b],
                "linvE": linvE[b],
                "lamE": lamE[b],
            }
        )
    res = run_bass_kernel_spmd(nc, in_maps, core_ids=list(range(B)))
    if _results_hook is not None:
        _results_hook(res)
    out = np.stack([res.results[b]["out"] for b in range(B)], axis=0)
    return out.astype(np.float32)
